# revision 2
# baseline (speedup 1.0000x reference)
"""Trainium2 Bass kernel v4 for MetaPathClassifier.

Per-core design (512 paths, sorted by length descending, bf16 throughout):
  - Union feature table padded to 224-elem rows ([paper128|author64|venue32]
    at type-specific offsets, zeros elsewhere): a slot's gather IS its
    masked union feature row -> no on-chip split/masking at all.
  - Gathers: one indirect DMA per 128 active slots, slots packed densely
    across step boundaries (20 instructions); Pool engine is dedicated to
    SWDGE descriptor generation (~1us/instr) plus light gate work.
  - PE transposes per block -> per-step PSUM tile; gT copies on DVE,
    xn PSUM->SBUF copies ride the idle SP DMA queue (f32, no convert).
  - Per-step input-gate GEMM accumulates into the same PSUM tile as the
    recurrent GEMM; b_ih (all gates) and b_hh (r/z only) are folded into
    the one-hot aux rows so r/z sigmoids read PSUM with no bias operand.
    BIGZ on the invalid aux row freezes finished paths exactly.
"""

import numpy as np

import concourse.bacc as bacc
import concourse.bass as bass
import concourse.mybir as mybir
import concourse.tile as tile
from concourse.bass import IndirectOffsetOnAxis
from concourse.bass_utils import run_bass_kernel_spmd
from concourse.masks import make_identity

F32 = mybir.dt.float32
BF16 = mybir.dt.bfloat16
I32 = mybir.dt.int32
AF = mybir.ActivationFunctionType
OP = mybir.AluOpType

NCORES = 8
B, L, H, C = 4096, 8, 256, 8
NB = B // NCORES
S = NB * L
G = 3 * H
NM = G // 128
NP, DP = 600000, 128
NA, DA = 600000, 64
NV, DV = 100000, 32
KAV = 100
PW = 256                     # padded union row width (elems)
NNODES = NP + NA + NV
UR4 = (NNODES + 1) * (PW // 32)   # +1 ghost row for invalid slots
BIGZ = 30000.0
NJ = 32                      # offu column capacity


def _plan(nl):
    """Concat-packed gather blocks and their gT segments."""
    cum = np.concatenate([[0], np.cumsum(nl)]).astype(int)
    tot = int(cum[-1])
    nblk = (tot + 127) // 128
    bend = [0] * L
    bstart = [0] * L
    for l in range(L):
        bend[l] = (cum[l + 1] + 127) // 128
    prev = 0
    for l in range(L):
        bstart[l] = prev
        prev = bend[l]
    # segment list per step: (block, local_col, gcol, width) covering
    # the step's active columns from the blocks' transposed quadrants
    segs = {l: [] for l in range(L)}
    for l in range(L):
        a, b = int(cum[l]), int(cum[l + 1])
        g = a
        while g < b:
            blk = g // 128
            loc = g - blk * 128
            w = min(b - g, 128 - loc)
            segs[l].append((blk, loc, g - a, w))
            g += w
    return cum, tot, nblk, bstart, bend, segs


def build_nc(nl, taps=False, reps=1, loop_n=0):
    nl = tuple(nl)
    cum, tot, nblk, bstart, bend, segs = _plan(nl)
    steps = [l for l in range(L) if nl[l] > 0]

    nc = bacc.Bacc("TRN2", target_bir_lowering=False, debug=False,
                   num_devices=NCORES)

    def din(name, shape, dt=F32):
        return nc.dram_tensor(name, shape, dt, kind="ExternalInput").ap()

    ux_d = din("ux", [UR4, 32], BF16)
    wc1_d = din("wc1", [DP, G], BF16)
    lhs2_d = din("lhs2", [KAV, G], BF16)
    whh_d = din("whhT", [H, G], BF16)
    wct_d = din("wcT", [H, C], BF16)
    offs_d = din("offu", [128, NJ], I32)
    bhh6_d = din("bhh6", [128, NM])
    bc_d = din("bc8", [C, 1])
    out_d = nc.dram_tensor("logitsT", [C, NB], F32, kind="ExternalOutput").ap()

    with tile.TileContext(nc) as tc:
        pers = tc.alloc_tile_pool(name="pers", bufs=1)

        def T(shape, dt, name):
            return pers.tile(shape, dt, tag=name, name=name)

        gTa = T([128, S], BF16, name="gTa")     # paper-part rows 0:128
        gTb = T([KAV, S], BF16, name="gTb")     # av rows 0:96 + aux 96:100
        wc1s = T([DP, G], BF16, name="wc1s")
        lhs2s = T([KAV, G], BF16, name="lhs2s")
        swhh = [T([128, G], BF16, name=f"swhh{k}") for k in range(2)]
        swc = [T([128, C], BF16, name=f"swc{k}") for k in range(2)]
        soff = T([128, NJ], I32, name="soff")
        bhh6 = T([128, NM], F32, name="bhh6")
        sbc = T([C, 1], F32, name="sbc")
        hT = [T([128, NB], BF16, name=f"hT{k}") for k in range(2)]
        ident = T([128, 128], BF16, name="ident")
        lsb = T([C, NB], F32, name="lsb")
        ub = [T([128, PW], BF16, name=f"u{b_}") for b_ in range(nblk)]
        # (table cols 224:227 carry the per-node one-hot; col 227 is the
        # ghost-row invalid flag that lands on the BIGZ row of lhs2)

        with (
            tc.tile_pool(name="psT", bufs=2, space="PSUM") as psT,
            tc.tile_pool(name="psG", bufs=6, space="PSUM") as psG,
            tc.tile_pool(name="gate", bufs=2) as gp,
        ):
            # SP queue: offsets first (gathers wait on them), aux + weights
            nc.sync.dma_start(soff[:], offs_d[:, :])
            nc.sync.dma_start(wc1s[:], wc1_d[:, :])
            nc.sync.dma_start(lhs2s[:], lhs2_d[:, :])
            for k in range(2):
                hs = slice(k * 128, (k + 1) * 128)
                nc.sync.dma_start(swhh[k][:], whh_d[hs, :])
            nc.sync.dma_start(bhh6[:], bhh6_d[:, :])
            for k in range(2):
                hs = slice(k * 128, (k + 1) * 128)
                nc.sync.dma_start(swc[k][:], wct_d[hs, :])
            nc.sync.dma_start(sbc[:], bc_d[:, :])
            # Pool: identity (needed by first transpose) before gathers
            make_identity(nc, ident[:])
            if nl[0] < NB:
                nc.gpsimd.memset(hT[0][:], 0.0)
                nc.gpsimd.memset(hT[1][:], 0.0)

            def gather(b_):
                nc.gpsimd.indirect_dma_start(
                    out=ub[b_][:, :], out_offset=None,
                    in_=ux_d[:, :],
                    in_offset=IndirectOffsetOnAxis(
                        ap=soff[:, b_:b_ + 1], axis=0),
                    bounds_check=UR4 - 1, oob_is_err=False)

            state = {"gath_done": 0}

            def gather_upto(b_):
                while state["gath_done"] < min(b_, nblk):
                    gather(state["gath_done"])
                    state["gath_done"] += 1

            pg_rz = {}
            xn_sb = {}
            pg_nh = {}
            gates = {}

            _rep = 0

            def prep_a(l):
                n = nl[l]
                c0 = l * NB
                nb_ = bend[l] - bstart[l]
                tp = psT.tile([128, 1024], BF16, tag="t", name=f"tp{_rep}_{l}")
                for j, b_ in enumerate(range(bstart[l], bend[l])):
                    nc.tensor.transpose(tp[:, j * 256:j * 256 + 128],
                                        ub[b_][:, 0:128], ident[:])
                    nc.tensor.transpose(tp[0:100, j * 256 + 128:j * 256 + 256],
                                        ub[b_][:, 128:228], ident[:])
                # gT copies (DVE, bf16): per segment of this step
                for (blk, loc, gc, w) in segs[l]:
                    j = blk - bstart[l]
                    if j < 0:      # boundary block processed by prior step
                        jtp = prev_tp[0]
                        j = blk - prev_bs[0]
                    else:
                        jtp = tp
                    nc.vector.tensor_copy(
                        gTa[:, c0 + gc:c0 + gc + w],
                        jtp[:, j * 256 + loc:j * 256 + loc + w])
                    nc.vector.tensor_copy(
                        gTb[0:100, c0 + gc:c0 + gc + w],
                        jtp[0:100, j * 256 + 128 + loc:j * 256 + 128 + loc + w])
                prev_tp[0] = tp
                prev_bs[0] = bstart[l]
                cs = slice(c0, c0 + n)
                for mt in range(2):
                    m = 4 + mt
                    ms = slice(m * 128, (m + 1) * 128)
                    px = psG.tile([128, NB], F32, tag="g", name=f"xn{_rep}_{l}_{mt}")
                    nc.tensor.matmul(px[:, 0:n], wc1s[:, ms], gTa[:, cs],
                                     start=True, stop=False)
                    nc.tensor.matmul(px[:, 0:n], lhs2s[:, ms], gTb[:, cs],
                                     start=False, stop=True)
                    xs = gp.tile([128, NB], BF16, tag="xn", bufs=4,
                                 name=f"xs{_rep}_{l}_{mt}")
                    if mt == 0:
                        nc.scalar.copy(xs[:, 0:n], px[:, 0:n])
                    else:
                        nc.vector.tensor_copy(xs[:, 0:n], px[:, 0:n])
                    xn_sb[(l, mt)] = xs

            prev_tp = [None]
            prev_bs = [None]

            def prep_b(l):
                n = nl[l]
                cs = slice(l * NB, l * NB + n)
                last = (l == 0)
                for m in range(4):
                    ms = slice(m * 128, (m + 1) * 128)
                    pg = psG.tile([128, NB], F32, tag="g", name=f"rz{_rep}_{l}_{m}")
                    nc.tensor.matmul(pg[:, 0:n], wc1s[:, ms], gTa[:, cs],
                                     start=True, stop=False,
                                     skip_group_check=True)
                    nc.tensor.matmul(pg[:, 0:n], lhs2s[:, ms], gTb[:, cs],
                                     start=False, stop=last,
                                     skip_group_check=True)
                    pg_rz[(l, m)] = pg

            def gru_a1(l):
                if l == 0:
                    return
                n = nl[l]
                for m in (0, 1, 2, 3):
                    po = pg_rz[(l, m)][:, 0:n]
                    ms = slice(m * 128, (m + 1) * 128)
                    for k in range(2):
                        nc.tensor.matmul(po, swhh[k][:, ms], hT[k][:, 0:n],
                                         start=False, stop=(k == 1),
                                         skip_group_check=True)

            def gru_b1(l):
                n = nl[l]
                rr, zz = [], []
                for mt in range(2):
                    r = gp.tile([128, NB], BF16, tag="rr", bufs=2,
                                name=f"rr{_rep}_{l}{mt}")
                    nc.scalar.activation(r[:, 0:n], pg_rz[(l, mt)][:, 0:n],
                                         AF.Sigmoid)
                    rr.append(r)
                for mt in range(2):
                    z = gp.tile([128, NB], BF16, tag="zz", bufs=2,
                                name=f"zz{_rep}_{l}{mt}")
                    nc.scalar.activation(z[:, 0:n], pg_rz[(l, 2 + mt)][:, 0:n],
                                         AF.Sigmoid, scale=-1.0)
                    zz.append(z)
                gates[l] = (rr, zz)

            def gru_a2(l):
                if l == 0:
                    return
                n = nl[l]
                for mt in range(2):
                    ph = psG.tile([128, NB], F32, tag="g", name=f"nh{_rep}_{l}_{mt}")
                    pg_nh[(l, mt)] = ph
                    ms = slice((4 + mt) * 128, (5 + mt) * 128)
                    for k in range(2):
                        nc.tensor.matmul(ph[:, 0:n], swhh[k][:, ms],
                                         hT[k][:, 0:n],
                                         start=(k == 0), stop=(k == 1))

            def gru_b2(l):
                n = nl[l]
                rr, zz = gates.pop(l)
                tts = []
                for mt in range(2):
                    t_ = gp.tile([128, NB], F32, tag="tt", name=f"tt{_rep}_{l}{mt}")
                    if l > 0:
                        nc.vector.scalar_tensor_tensor(
                            t_[:, 0:n], pg_nh[(l, mt)][:, 0:n],
                            bhh6[:, 4 + mt:5 + mt], rr[mt][:, 0:n],
                            op0=OP.add, op1=OP.mult)
                    else:
                        nc.vector.tensor_scalar_mul(
                            t_[:, 0:n], rr[mt][:, 0:n],
                            bhh6[:, 4 + mt:5 + mt])
                    tts.append(t_)
                nps, nns = [], []
                for mt in range(2):
                    np_ = gp.tile([128, NB], F32, tag="npre", name=f"np{_rep}_{l}{mt}")
                    nc.vector.tensor_add(np_[:, 0:n],
                                         xn_sb[(l, mt)][:, 0:n],
                                         tts[mt][:, 0:n])
                    nps.append(np_)
                for mt in range(2):
                    nn_ = gp.tile([128, NB], BF16, tag="nn", name=f"nn{_rep}_{l}{mt}")
                    nc.scalar.activation(nn_[:, 0:n], nps[mt][:, 0:n], AF.Tanh)
                    nns.append(nn_)
                if l == 0:
                    for mt in range(2):
                        nc.vector.tensor_mul(hT[mt][:, 0:n], zz[mt][:, 0:n],
                                             nns[mt][:, 0:n])
                    return
                dds, ees = [], []
                for mt in range(2):
                    d = gp.tile([128, NB], BF16, tag="dd", name=f"dd{_rep}_{l}{mt}")
                    nc.gpsimd.tensor_sub(d[:, 0:n], nns[mt][:, 0:n],
                                         hT[mt][:, 0:n])
                    dds.append(d)
                for mt in range(2):
                    e = gp.tile([128, NB], BF16, tag="ee", name=f"ee{_rep}_{l}{mt}")
                    nc.gpsimd.tensor_mul(e[:, 0:n], zz[mt][:, 0:n],
                                         dds[mt][:, 0:n])
                    ees.append(e)
                for mt in range(2):
                    nc.vector.tensor_add(hT[mt][:, 0:n], hT[mt][:, 0:n],
                                         ees[mt][:, 0:n])

            import contextlib

            def _iter_ctx():
                if loop_n:
                    return tc.For_i(0, loop_n, name="outer")
                return contextlib.nullcontext(0)

            with _iter_ctx() as _iv_or_none:
                _nreps = 1 if loop_n else reps
                for _rep in range(_nreps):
                    state["gath_done"] = 0
                    # prefetch the first two steps' blocks
                    gather_upto(bend[steps[min(1, len(steps) - 1)]])
                    for i, l in enumerate(steps):
                        if i >= 1:
                            p = steps[i - 1]
                            gru_a1(p)
                            gru_b1(p)
                            gru_a2(p)
                            gru_b2(p)
                        if i + 1 < len(steps):
                            gather_upto(bend[steps[i + 1]])
                        prep_a(l)
                        prep_b(l)
                    gather_upto(nblk)
                    p = steps[-1]
                    gru_a1(p)
                    gru_b1(p)
                    gru_a2(p)
                    gru_b2(p)

                    pl = psG.tile([128, NB], F32, tag="g", name=f"plog{_rep}")
                    for k in range(2):
                        nc.tensor.matmul(pl[0:C, :], swc[k][:], hT[k][:],
                                         start=(k == 0), stop=(k == 1))
                    nc.vector.tensor_scalar(lsb[:], pl[0:C, :], sbc[:, 0:1],
                                            None, op0=OP.add)
                    nc.sync.dma_start(out_d[:, :], lsb[:])

        pers.release()

    nc.finalize()
    return nc


_NC_CACHE = {}
_LAST_NL = None


def _get_nc(nl=None):
    global _LAST_NL
    if nl is None:
        nl = _LAST_NL
    assert nl is not None
    nl = tuple(nl)
    _LAST_NL = nl
    if nl not in _NC_CACHE:
        _NC_CACHE[nl] = build_nc(nl)
    return _NC_CACHE[nl]


def _compute_nl(lengths):
    counts = np.zeros(L, np.int64)
    for c in range(NCORES):
        lens = np.asarray(lengths[c * NB:(c + 1) * NB])
        for l in range(L):
            counts[l] = max(counts[l], int((lens > l).sum()))
    return tuple(min(NB, int(-(-c_ // 32) * 32)) for c_ in counts)


_UX_CACHE = {}


def _build_ux(inputs):
    key = id(inputs.get("paper_x"))
    if key in _UX_CACHE:
        return _UX_CACHE[key]
    import ml_dtypes
    f = lambda k: np.asarray(inputs[k], dtype=np.float32)
    ux = np.zeros((NNODES + 1, PW), ml_dtypes.bfloat16)
    ux[0:NP, 0:DP] = f("paper_x")
    ux[NP:NP + NA, DP:DP + DA] = f("author_x")
    ux[NP + NA:NNODES, DP + DA:DP + DA + DV] = f("venue_x")
    ux[0:NP, 224] = 1.0
    ux[NP:NP + NA, 225] = 1.0
    ux[NP + NA:NNODES, 226] = 1.0
    ux[NNODES, 227] = 1.0          # ghost row: invalid-slot freeze flag
    ux = np.ascontiguousarray(ux.reshape(UR4, 32))
    _UX_CACHE.clear()
    _UX_CACHE[key] = ux
    return ux


def make_in_maps(inputs):
    global _LAST_NL
    import ml_dtypes
    f = lambda k: np.asarray(inputs[k], dtype=np.float32)
    bf = lambda a: np.ascontiguousarray(a.astype(ml_dtypes.bfloat16))

    W_ih, W_hh = f("W_ih"), f("W_hh")
    b_ih, b_hh = f("b_ih"), f("b_hh")

    wc1 = (W_ih @ f("Wp")).T
    lhs2 = np.zeros((KAV, G), np.float32)
    lhs2[0:DA] = (W_ih @ f("Wa")).T
    lhs2[DA:DA + DV] = (W_ih @ f("Wv")).T
    B3 = np.stack([f("bp"), f("ba"), f("bv")], axis=1)
    # one-hot aux rows sum to 1 per slot: fold b_ih (all gates) and b_hh
    # (r/z gates only; the n-gate's b_hh rides inside r*(hn+b_hh)) into them
    fold = b_ih.copy()
    fold[0:2 * H] += b_hh[0:2 * H]
    lhs2[96:99] = (W_ih @ B3).T + fold[None, :]
    lhs2[99, H:2 * H] = BIGZ

    shared = {
        "ux": _build_ux(inputs),
        "wc1": bf(wc1),
        "lhs2": bf(lhs2),
        "whhT": bf(np.ascontiguousarray(W_hh.T)),
        "wcT": bf(np.ascontiguousarray(f("Wc").T)),
        "bhh6": np.ascontiguousarray(b_hh.reshape(NM, 128).T),
        "bc8": f("bc").reshape(C, 1),
    }

    type_ids = np.asarray(inputs["type_ids"], dtype=np.int64)
    node_ids = np.asarray(inputs["node_ids"], dtype=np.int64)
    lengths = np.asarray(inputs["lengths"], dtype=np.int64)
    nl = _compute_nl(lengths)
    _LAST_NL = nl
    cum, tot, nblk, bstart, bend, segs = _plan(nl)

    in_maps = []
    perms = []
    for c in range(NCORES):
        sl = slice(c * NB, (c + 1) * NB)
        lens = lengths[sl]
        perm = np.argsort(-lens, kind="stable")
        perms.append(perm)
        lens_s = lens[perm]
        tid_s = type_ids[sl][perm]
        nid_s = node_ids[sl][perm]

        offu = np.zeros((128, NJ), np.int32)
        for l in range(L):
            t_l = tid_s[:, l].astype(np.int64)
            i_l = nid_s[:, l].astype(np.int64)
            base = np.where(t_l == 0, 0,
                            np.where(t_l == 1, NP, NP + NA))
            rows = (base + i_l) * (PW // 32)
            rows = np.where(l >= lens_s, NNODES * (PW // 32), rows)
            n = nl[l]
            gg = cum[l] + np.arange(n)
            offu[gg % 128, gg // 128] = rows[0:n]
        m = dict(shared)
        m["offu"] = np.ascontiguousarray(offu)
        in_maps.append(m)
    return in_maps, perms


def kernel(**inputs) -> np.ndarray:
    in_maps, perms = make_in_maps(inputs)
    nc = _get_nc()
    res = run_bass_kernel_spmd(nc, in_maps, core_ids=list(range(NCORES)))
    outs = []
    for c in range(NCORES):
        lt = np.asarray(res.results[c]["logitsT"]).T
        un = np.empty_like(lt)
        un[perms[c]] = lt
        outs.append(un)
    return np.ascontiguousarray(
        np.concatenate(outs, axis=0).astype(np.float32))

